# revision 2
# baseline (speedup 1.0000x reference)
"""Trainium2 Bass kernel v2 for the FISTA sparse-coding encoder.

Math per iteration (identical to reference, rank-16 form):
    r   = Y - D @ y            (u-pass, PE; rhs = bf16 truncation-view of y)
    zps = Dsc^T @ r            (v-pass, PE; Dsc stored as stacked bf16 hi+lo
                                pair on partitions 0-15/16-31, contracting 32)
    x~  = beta_i * softshrink(zps + y, lambd)   (ONE custom DVE op, fp32)
    y   = x~_new - gamma_i * x~_old             (stt split DVE/Pool, fp32)

Precision design (validated in numpy, rel_l2 ~ 4e-3 vs 2e-2 budget):
 - state (x~, y, z) fp32: FISTA steps are ~linv=2e-4 scale, far below bf16.
 - matmul inputs bf16: y and rsa are read through strided APs selecting the
   high halfword of each fp32 word (= bf16 truncation) -- zero cast cost.
 - Dsc as bf16 hi+lo (its rounding bias competes with lambd ~ 2e-5).

Parallelism: 2 independent 128-pixel blocks per core pipeline the
per-iteration dependency cycle  ssadd -> y -> u -> rsa -> v -> ssadd.
"""

from contextlib import ExitStack

import numpy as np
import ml_dtypes

import concourse.bass as bass
import concourse.bacc as bacc
import concourse.mybir as mybir
import concourse.tile as tile
from concourse.bass_utils import run_bass_kernel_spmd

BF16NP = ml_dtypes.bfloat16

T = 16
NPOLE = 161
K = 4 * NPOLE          # 644
KPAD = 768             # 6 k-tiles of 128
NKT = 6
P_TOTAL = 2048
N_CORES = 8
P_SHARD = P_TOTAL // N_CORES   # 256
NBLK = 2
PB = P_SHARD // NBLK           # 128 pixels per block
MAXITER = 100
LAM = np.float32(0.1)

FP32 = mybir.dt.float32
BF16 = mybir.dt.bfloat16
AF = mybir.ActivationFunctionType
ALU = mybir.AluOpType

# bf16 weight pack layout [128, C16T]
C_NDT = 0                        # -D^T hi k-tiles [128, 32] (2-replica cols)
C_DSC = C_NDT + 32 * NKT         # Dsc stacked hi/lo [32, 768]
C16T = C_DSC + KPAD
# fp32 pack layout [128, C32T]
C_Y = 0                          # Y shard [16, 256]
C_I16 = C_Y + P_SHARD            # I16 2-replica [16, 32]
C32T = C_I16 + 32


# ---- custom DVE op: out = imm2 * ((in0+in1) - clip(in0+in1, [s1, s0])) ----
def _register_softshrink():
    import concourse.dve_ops as dve_ops_mod
    from concourse.dve_ops import DveOp
    from concourse.dve_spec import C0, C1, C2, Spec, Src0, Src1, maxx, minn

    name = "SOFTSHRINK_ADD_ANT"
    for op in dve_ops_mod.OPS:
        if op.name == name:
            return op

    def _ref(in0, in1, s0, s1, imm2):
        z = in0 + in1
        return ((z - np.maximum(np.minimum(z, s0), s1)) * imm2).astype(in0.dtype)

    z = Src0 + Src1
    spec = Spec(body=(z - maxx(minn(z, C0), C1)) * C2, reference=_ref)
    op = DveOp(
        name,
        spec,
        subdim=False,
        uops_sha={"v3": "426f16d475f55128", "v4": "fa84f9d3d75bd94f"},
    )
    dve_ops_mod.OPS.append(op)
    dve_ops_mod._SUB_OPCODE_FOR_NAME[name] = (
        dve_ops_mod._CUSTOM_DVE_ROW_BASE + len(dve_ops_mod.OPS) - 1
    )
    dve_ops_mod.CUSTOM_DVE_SPECS[name] = spec
    return op


SOFTSHRINK_OP = _register_softshrink()


def _bf16_view(ap_f32):
    """Strided bf16 view selecting the high halfword of each fp32 element
    (little-endian: halfword index 1) == bf16 truncation of the value."""
    v = ap_f32.bitcast(BF16)
    return v.rearrange("p (c two) -> p c two", two=2)[:, :, 1]


def _build_dictionary_np(Drr, Dtheta):
    i = np.arange(T, dtype=np.float32)[:, None]
    pr = Drr[None, :] ** i
    sgn = (np.float32(-1.0)) ** i
    c = np.cos(i * Dtheta[None, :])
    s = np.sin(i * Dtheta[None, :])
    dic = np.concatenate([pr * c, sgn * pr * c, pr * s, sgn * pr * s], axis=1).astype(
        np.float32
    )
    mean = dic.mean(axis=0, keepdims=True, dtype=np.float32).astype(np.float32)
    dic = dic - mean
    std = dic.std(axis=0, ddof=1, keepdims=True).astype(np.float32)
    std = np.where(std == 0, np.ones_like(std), std)
    return (dic / std).astype(np.float32)


def _host_precompute(Drr, Dtheta, n_iter=MAXITER):
    D = _build_dictionary_np(Drr.astype(np.float32), Dtheta.astype(np.float32))
    DtD = (D.T @ D).astype(np.float32)
    L = np.float32(np.linalg.norm(DtD))
    linv = np.float32(1.0) / L
    lambd = np.float32(LAM * linv)

    tts = []
    t = np.float32(1.0)
    for _ in range(n_iter):
        t_new = (
            np.float32(1.0)
            + np.sqrt(np.float32(1.0) + np.float32(4.0) * t * t, dtype=np.float32)
        ) / np.float32(2.0)
        tts.append(np.float32((t - np.float32(1.0)) / t_new))
        t = t_new
    tts = np.array(tts, dtype=np.float32)
    betas = (np.float32(1.0) + tts).astype(np.float32)
    betas[n_iter - 1] = np.float32(1.0)
    gammas = np.zeros(n_iter, np.float32)
    for i in range(1, n_iter):
        gammas[i] = np.float32(tts[i] / betas[i - 1])

    Dpad = np.zeros((T, KPAD), np.float32)
    Dpad[:, :K] = D
    Dsc = (Dpad * linv).astype(np.float32)

    Dhi = Dpad.astype(BF16NP)                       # bf16 RN of D
    DscHi = Dsc.astype(BF16NP)
    DscLo = (Dsc - DscHi.astype(np.float32)).astype(BF16NP)

    w16 = np.zeros((128, C16T), BF16NP)
    for j in range(NKT):
        for g in range(2):
            w16[:, C_NDT + 32 * j + 16 * g : C_NDT + 32 * j + 16 * (g + 1)] = -Dhi.T[
                128 * j : 128 * (j + 1), :
            ]
    w16[0:T, C_DSC : C_DSC + KPAD] = DscHi
    w16[T : 2 * T, C_DSC : C_DSC + KPAD] = DscLo

    w32c = np.zeros((128, C32T), np.float32)
    for g in range(2):
        w32c[:T, C_I16 + 16 * g : C_I16 + 16 * (g + 1)] = np.eye(T, dtype=np.float32)
    return dict(
        lambd=lambd, tts=tts, betas=betas, gammas=gammas, D=D, linv=linv,
        w16=w16, w32c=w32c,
    )


def _pack_input(pc, y_shard):
    w32 = pc["w32c"].copy()
    w32[:T, C_Y : C_Y + P_SHARD] = y_shard
    return dict(wp32=w32, wp16=pc["w16"])


def _build_bass(pc, n_iter=MAXITER, n_reps=1, dynamic_reps=False,
                y_dve_cols=512, n_phase=2, ssadd_split=False, rsa_dve=True):
    """y_dve_cols: first columns of the y-update on DVE (right after the
    softshrink, same engine); the rest go to Pool."""
    lam = float(pc["lambd"])
    betas = pc["betas"]
    gammas = pc["gammas"]

    nc = bacc.Bacc("TRN2", target_bir_lowering=False, debug=False)

    d_w32 = nc.dram_tensor("wp32", [128, C32T], FP32, kind="ExternalInput").ap()
    d_w16 = nc.dram_tensor("wp16", [128, C16T], BF16, kind="ExternalInput").ap()
    d_out = nc.dram_tensor("out", [K, P_SHARD], FP32, kind="ExternalOutput").ap()

    with ExitStack() as ctx, tile.TileContext(nc) as tc:
        s_w32 = nc.alloc_sbuf_tensor("s_w32", [128, C32T], FP32).ap()
        s_w16 = nc.alloc_sbuf_tensor("s_w16", [128, C16T], BF16).ap()
        s_i16 = s_w32[0:T, C_I16 : C_I16 + 32]

        blk = []
        for b in range(NBLK):
            d = dict(
                xa=nc.alloc_sbuf_tensor(f"xa{b}", [128, KPAD], FP32).ap(),
                xb=nc.alloc_sbuf_tensor(f"xb{b}", [128, KPAD], FP32).ap(),
                y=nc.alloc_sbuf_tensor(f"y{b}", [128, KPAD], FP32).ap(),
                rsa=nc.alloc_sbuf_tensor(f"rsa{b}", [32, PB], FP32).ap(),
                tg=nc.alloc_sbuf_tensor(f"tg{b}", [128, KPAD], FP32).ap(),
                rps=nc.alloc_psum_tensor(f"rps{b}", [32, PB], FP32).ap(),
                zps=nc.alloc_psum_tensor(f"zps{b}", [128, KPAD], FP32).ap(),
                yin=s_w32[0:T, C_Y + b * PB : C_Y + (b + 1) * PB],
            )
            blk.append(d)

        nc.sync.dma_start(s_w32, d_w32)
        nc.sync.dma_start(s_w16, d_w16)

        import contextlib

        def rep_ctx():
            if dynamic_reps and n_reps > 1:
                return tc.For_i(0, n_reps, 1)
            return contextlib.nullcontext(0)

        for rep in range(1 if dynamic_reps else n_reps):
          with rep_ctx() as _iv:
            for b in range(NBLK):
                nc.vector.memset(blk[b]["xa"], 0.0)
                nc.vector.memset(blk[b]["y"], 0.0)

            for i in range(n_iter):
                beta = float(betas[i])
                gamma = float(gammas[i])
                last = i == n_iter - 1
                x_old = {}
                x_new = {}
                for b in range(NBLK):
                    s = blk[b]
                    x_old[b] = s["xa"] if i % 2 == 0 else s["xb"]
                    x_new[b] = s["xb"] if i % 2 == 0 else s["xa"]

                # off-path: Pool precomputes t = -gamma * x~_old for the y tail
                if not last and y_dve_cols < KPAD:
                    for b in range(NBLK):
                        nc.gpsimd.tensor_scalar(
                            blk[b]["tg"][:, y_dve_cols:KPAD],
                            x_old[b][:, y_dve_cols:KPAD],
                            -gamma, 0.0, ALU.mult, ALU.add,
                        )

                # u-pass: rps = I16 @ Y - sum_j D_j^T ybview_j  ([32,128], 2-rep)
                for b in range(NBLK):
                    s = blk[b]
                    nc.tensor.matmul(
                        s["rps"], s_i16, s["yin"], start=True, stop=(i == 0),
                    )
                    if i != 0:
                        yv = _bf16_view(s["y"])
                        for j in range(NKT):
                            nc.tensor.matmul(
                                s["rps"],
                                s_w16[:, C_NDT + 32 * j : C_NDT + 32 * (j + 1)],
                                yv[:, PB * j : PB * (j + 1)],
                                start=False,
                                stop=(j == NKT - 1),
                            )

                for b in range(NBLK):
                    cp = (nc.vector.tensor_copy if rsa_dve else nc.scalar.copy)
                    if i == 0 and b == 1:
                        # phase-shift block 1's chain by ~half an iteration:
                        # WAW dummy copies delay its first v-pass; the offset
                        # then self-sustains across iterations
                        for _ in range(n_phase):
                            cp(blk[b]["rsa"], blk[b]["rps"])
                    cp(blk[b]["rsa"], blk[b]["rps"])

                # v-pass: zps_j = [DscHi_j; DscLo_j]^T @ [rsa; rsa]
                for b in range(NBLK):
                    s = blk[b]
                    rv = _bf16_view(s["rsa"])
                    for j in range(NKT):
                        nc.tensor.matmul(
                            s["zps"][:, PB * j : PB * (j + 1)],
                            s_w16[0 : 2 * T, C_DSC + 128 * j : C_DSC + 128 * (j + 1)],
                            rv,
                            start=True,
                            stop=True,
                        )

                # x~ = beta * softshrink(zps + y); then y-update for the same
                # block immediately (DVE cols in program order => no sem, and
                # the other block's ssadd doesn't delay this block's y)
                hc = y_dve_cols
                for b in range(NBLK):
                    if ssadd_split and hc < KPAD:
                        # tail cols first: unblocks the Pool y-update early
                        for lo, hi in ((hc, KPAD), (0, hc)):
                            nc.vector._custom_dve(
                                SOFTSHRINK_OP,
                                out=x_new[b][:, lo:hi],
                                in0=blk[b]["zps"][:, lo:hi],
                                in1=blk[b]["y"][:, lo:hi],
                                s0=lam,
                                s1=-lam,
                                imm2=beta,
                            )
                    else:
                        nc.vector._custom_dve(
                            SOFTSHRINK_OP,
                            out=x_new[b],
                            in0=blk[b]["zps"],
                            in1=blk[b]["y"],
                            s0=lam,
                            s1=-lam,
                            imm2=beta,
                        )
                    if not last:
                        if hc < KPAD:
                            # t was precomputed off-path; y tail = t + x~_new
                            nc.gpsimd.tensor_tensor(
                                blk[b]["y"][:, hc:KPAD], blk[b]["tg"][:, hc:KPAD],
                                x_new[b][:, hc:KPAD], ALU.add,
                            )
                        if hc > 0:
                            nc.vector.scalar_tensor_tensor(
                                blk[b]["y"][:, 0:hc], x_old[b][:, 0:hc], -gamma,
                                x_new[b][:, 0:hc], ALU.mult, ALU.add,
                            )

        for b in range(NBLK):
            s = blk[b]
            x_fin = s["xb"] if (n_iter - 1) % 2 == 0 else s["xa"]
            for j in range(NKT):
                rows = min(128, K - 128 * j)
                if rows <= 0:
                    break
                nc.sync.dma_start(
                    d_out[128 * j : 128 * j + rows, b * PB : (b + 1) * PB],
                    x_fin[0:rows, PB * j : PB * j + PB],
                )
    nc.compile()
    return nc


_CACHE = {}


def kernel(Drr, Dtheta, x):
    pc = _host_precompute(np.asarray(Drr), np.asarray(Dtheta))
    if "nc" not in _CACHE:
        _CACHE["nc"] = _build_bass(pc)
    nc = _CACHE["nc"]

    xf = np.asarray(x, np.float32)  # [1, 16, 2048]
    in_maps = [
        _pack_input(pc, xf[0, :, c * P_SHARD : (c + 1) * P_SHARD])
        for c in range(N_CORES)
    ]
    res = run_bass_kernel_spmd(nc, in_maps, list(range(N_CORES)))
    out = np.zeros((1, K, P_TOTAL), np.float32)
    for c in range(N_CORES):
        out[0, :, c * P_SHARD : (c + 1) * P_SHARD] = res.results[c]["out"]
    return out
